# revision 1
# baseline (speedup 1.0000x reference)
"""Trainium2 Bass kernel for nn_CMAAA_29274497089816 (sparse local attention).

Sharding: data-parallel B(2) x H-slab(4) over 8 cores. Each core handles one
batch sample and a 64-row output slab. Host prepares padded input slabs,
folded conv weights (cond/s and pan-lpan folds baked in), and the scrambled
k_ms "S" field (small band conv in numpy); the chip runs the big convs and
the full neighborhood attention.
"""
import sys, os
sys.path.insert(0, "/opt/trn_rl_repo")
import numpy as np
import ml_dtypes

import concourse.bass as bass
import concourse.bacc as bacc
import concourse.mybir as mybir
from concourse import tile
from concourse.bass_utils import run_bass_kernel_spmd

BF16 = mybir.dt.bfloat16
F32 = mybir.dt.float32
AF = mybir.ActivationFunctionType
ALU = mybir.AluOpType

DIM, HEADS, KA, MS_C, B, H, W = 32, 8, 3, 8, 2, 256, 256
HD, KK = 4, 9
SCALE = HD ** -0.5

NROW = 66            # field rows r0-1 .. r1+1
WP = 258             # padded width
NF = NROW * WP       # 17028 field pixels
FM = 2               # front/back margin elems in field tiles
NBLK = 4             # attention row-blocks per core
BR = 16              # out rows per block
PGRID = BR * WP      # 4128 real product px per block
NCH = 9              # chunks per block (9*512 = 4608 >= 4128)
CH = 512
PF = NCH * CH        # 4608 padded product px
RMARG = 2 * WP + 2   # replica tile read margin
RLEN = 20 * WP + 8


def _np(x):
    return np.ascontiguousarray(x)


# ---------------------------------------------------------------- host prep
def _fold_main_weights(w_q, w_kvms, w_vpan, sb):
    """lhsT_main[9, 51, 128]: channels [x32, msQ8, lpanQ1, ms8, lpan1, pan1],
    outputs [q(scaled)32, k_ms32, v_ms32, v_pan32]."""
    Ls = np.zeros((9, 51, 128), np.float32)
    i = 0
    for dy in range(3):
        for dx in range(3):
            L = Ls[i]; i += 1
            Wq = w_q[:, :, dy, dx]
            L[0:32, 0:32] = Wq[:, 0:32].T * SCALE
            L[32:40, 0:32] = Wq[:, 32:40].T * SCALE * sb
            L[40, 0:32] = Wq[:, 32:40].sum(1) * SCALE * (1.0 - sb)
            Wk = w_kvms[:, :, dy, dx]
            L[0:32, 32:64] = Wk[0:32, 0:32].T
            L[41:49, 32:64] = Wk[0:32, 32:40].T
            L[0:32, 64:96] = Wk[32:64, 0:32].T
            L[41:49, 64:96] = Wk[32:64, 32:40].T
            Wv = w_vpan[:, :, dy, dx]
            L[0:32, 96:128] = Wv[:, 0:32].T
            L[49, 96:128] = Wv[:, 32] - Wv[:, 34]
            L[50, 96:128] = Wv[:, 33] + Wv[:, 34]
    return Ls


def _attn_weights(w_dep, b_dep, w_proj_pan, b_proj_pan, w_proj_ms, b_proj_ms):
    Wd = np.zeros((4, 9, 9), np.float32)          # [d, t, j]
    for d in range(4):
        for j in range(9):
            Wd[d, :, j] = w_dep[d * 9 + j, 0].reshape(9)
    bd = b_dep.reshape(4, 9)                      # [d, j]

    # logits MM weights: lhsT_L[dy] [128, 72]; rows (dx,h,d) 0:96, q-rows 96:128
    L_L = np.zeros((3, 128, 72), np.float32)
    for dy in range(3):
        for dx in range(3):
            t = dy * 3 + dx
            for h in range(8):
                for d in range(4):
                    for j in range(9):
                        L_L[dy, dx * 32 + h * 4 + d, h * 9 + j] = Wd[d, t, j]
    for h in range(8):
        for d in range(4):
            for j in range(9):
                L_L[1, 96 + h * 4 + d, h * 9 + j] = bd[d, j]   # qb bias term

    # s0 sum MM: lhsT_s [72, 8]
    L_s = np.zeros((72, 8), np.float32)
    for h in range(8):
        L_s[h * 9:(h + 1) * 9, h] = 1.0
    # R72 broadcast MM: lhsT_R [8, 72]
    L_R = np.zeros((8, 72), np.float32)
    for h in range(8):
        L_R[h, h * 9:(h + 1) * 9] = 1.0
    # A MMs: lhsT_A[dy] [72, 128]: cols (dx,h,d) 0:96; dy==1 cols 96:128 = ba
    L_A = np.zeros((3, 72, 128), np.float32)
    for dy in range(3):
        for dx in range(3):
            t = dy * 3 + dx
            for h in range(8):
                for d in range(4):
                    for j in range(9):
                        L_A[dy, h * 9 + j, dx * 32 + h * 4 + d] = Wd[d, t, j]
    for h in range(8):
        for d in range(4):
            for j in range(9):
                L_A[1, h * 9 + j, 96 + h * 4 + d] = bd[d, j]
    # proj: lhsT_P[2, 128, 32]: rows (dx,h,d) = Wp.T replicated; rows 96:128 Wp.T
    L_P = np.zeros((2, 128, 32), np.float32)
    for bi, wp in enumerate([w_proj_pan, w_proj_ms]):
        wt = wp[:, :, 0, 0].T                     # [32in(h,d), 32out]
        for dx in range(3):
            L_P[bi, dx * 32:(dx + 1) * 32] = wt
        L_P[bi, 96:128] = wt
    pbias = np.stack([b_proj_pan, b_proj_ms]).reshape(2, 32, 1).astype(np.float32)
    return L_L, L_s, L_R, L_A, L_P, pbias


def _host_sfield(x, ms, w_kvms, b, r0):
    """Scrambled k_ms field [32,(h,d')], rows r0-1..r1+1, via numpy band conv."""
    R1 = r0 + 64
    need = {}
    for X in range(r0 - 1, R1 + 1):
        if 0 <= X < 256:
            need.setdefault(X % 4, set()).update(
                {64 * dp + X // 4 for dp in range(4)})
    cols = sorted(set().union(*need.values()))
    # conv inputs at cols +-1, all rows, zero padded
    xin = np.concatenate([x[b], ms[b]], 0)        # (40, 256, 256)
    xp = np.pad(xin, ((0, 0), (1, 1), (1, 1)))
    Wk = w_kvms[0:32]                             # k half (32, 40, 3, 3)
    kcols = np.zeros((32, 256, 256), np.float32)  # only needed cols filled
    for c in cols:
        acc = np.zeros((32, 256), np.float32)
        for dy in range(3):
            for dx in range(3):
                acc += np.einsum("oc,cy->oy", Wk[:, :, dy, dx],
                                 xp[:, dy:dy + 256, c + dx])
        kcols[:, :, c] = acc
    S = np.zeros((32, NROW, WP), np.float32)
    for hh in range(8):
        for dp in range(4):
            for gi, X in enumerate(range(r0 - 1, R1 + 1)):
                if 0 <= X < 256:
                    S[hh * 4 + dp, gi, 1:257] = \
                        kcols[hh * 4 + (X % 4), :, 64 * dp + X // 4]
    return S


# ---------------------------------------------------------------- bass build
_CACHE = {}


def _build_nc():
    if "nc" in _CACHE:
        return _CACHE["nc"]
    nc = bacc.Bacc(None, target_bir_lowering=False)
    FDL = 2 + NF + 524
    xin_d = nc.declare_dram_parameter("xin", [51, 68 * WP], BF16, isOutput=False)
    sf_d = nc.declare_dram_parameter("sfield", [32, FDL], BF16, isOutput=False)
    ones_d = nc.declare_dram_parameter("ones", [32, RLEN], BF16, isOutput=False)
    lm_d = nc.declare_dram_parameter("lhsT_main", [51, 9 * 128], BF16, isOutput=False)
    ll_d = nc.declare_dram_parameter("lhsT_L", [128, 3 * 72], BF16, isOutput=False)
    ls_d = nc.declare_dram_parameter("lhsT_s", [72, 8], BF16, isOutput=False)
    lr_d = nc.declare_dram_parameter("lhsT_R", [8, 72], BF16, isOutput=False)
    la_d = nc.declare_dram_parameter("lhsT_A", [72, 3 * 128], BF16, isOutput=False)
    lp_d = nc.declare_dram_parameter("lhsT_P", [128, 2 * 32], BF16, isOutput=False)
    pb_d = nc.declare_dram_parameter("pbias", [64, 1], F32, isOutput=False)
    mr_d = nc.declare_dram_parameter("rowmask", [128, 2], F32, isOutput=False)
    out_d = nc.declare_dram_parameter("out", [64, 64 * 256], F32, isOutput=True)

    with tile.TileContext(nc) as tc:
      with tc.sbuf_pool(name="persist", bufs=1) as pp:
        FT = 2 + NF + 524
        lm = pp.tile([51, 9 * 128], BF16, name="lm")
        nc.sync.dma_start(out=lm[:], in_=lm_d.ap())
        ll = pp.tile([128, 3 * 72], BF16, name="ll")
        nc.sync.dma_start(out=ll[:], in_=ll_d.ap())
        ls = pp.tile([72, 8], BF16, name="ls")
        nc.sync.dma_start(out=ls[:], in_=ls_d.ap())
        lr = pp.tile([8, 72], BF16, name="lr")
        nc.sync.dma_start(out=lr[:], in_=lr_d.ap())
        la = pp.tile([72, 3 * 128], BF16, name="la")
        nc.sync.dma_start(out=la[:], in_=la_d.ap())
        lp = pp.tile([128, 2 * 32], BF16, name="lp")
        nc.sync.dma_start(out=lp[:], in_=lp_d.ap())
        pb = pp.tile([64, 1], F32, name="pb")
        nc.sync.dma_start(out=pb[:], in_=pb_d.ap())
        mr = pp.tile([128, 2], F32, name="mr")
        nc.sync.dma_start(out=mr[:], in_=mr_d.ap())


        # ---------------- main convs ----------------
        dp = tc.alloc_tile_pool(name="fdp", bufs=1, space="DRAM")
        fdram = dp.tile([128, FT], BF16, name="fdram")
        with tc.sbuf_pool(name="convp", bufs=1) as cp, \
             tc.sbuf_pool(name="stg", bufs=4) as sgp, \
             tc.psum_pool(name="cpsum", bufs=3) as cps:
            xin = cp.tile([51, 68 * WP + 2], BF16, name="xin")
            NB = 1032
            for i in range(17):
                nc.sync.dma_start(out=xin[:, 1 + i * NB:1 + (i + 1) * NB],
                                  in_=xin_d.ap()[:, i * NB:(i + 1) * NB])
            nchunks = (NF + CH - 1) // CH
            for c in range(nchunks):
                base = c * CH
                n = min(CH, NF - base)
                ps = cps.tile([128, CH], F32, name="cps", tag="cps")
                it = 0
                for dy in range(3):
                    for dx in range(3):
                        nc.tensor.matmul(
                            ps[:, 0:n],
                            lm[:, it * 128:(it + 1) * 128],
                            xin[:, base + dy * WP + dx: base + dy * WP + dx + n],
                            start=(it == 0), stop=(it == 8))
                        it += 1
                st = sgp.tile([128, CH], BF16, name="st", tag="st")
                nc.vector.tensor_copy(st[:, 0:n], ps[:, 0:n])
                # zero the padded columns (y==0 and y==257 of each field row)
                w = ((base + WP - 1) // WP) * WP - base
                while w < n:
                    nc.vector.memset(st[:, w:w + 1], 0.0)
                    if w + WP - 1 < n:
                        nc.vector.memset(st[:, w + WP - 1:w + WP], 0.0)
                    w += WP
                wl = ((base + WP - 1) // WP) * WP - base - 1   # col 257 of prev row
                if 0 <= wl < n:
                    nc.vector.memset(st[:, wl:wl + 1], 0.0)
                # mask out-of-image top/bottom field rows (row 0 / row 65)
                if base == 0:
                    nc.vector.tensor_scalar_mul(st[:, 0:WP], st[:, 0:WP], mr[:, 0:1])
                r65a, r65b = 65 * WP, 66 * WP
                lo = max(base, r65a); hi = min(base + n, r65b)
                if lo < hi:
                    nc.vector.tensor_scalar_mul(st[:, lo - base:hi - base],
                                                st[:, lo - base:hi - base], mr[:, 1:2])
                nc.gpsimd.dma_start(out=fdram[:, 2 + base:2 + base + n],
                                    in_=st[:, 0:n])

        # ---------------- attention ----------------
        with tc.sbuf_pool(name="attn", bufs=2) as ap_, \
             tc.sbuf_pool(name="attn1", bufs=1) as ap1, \
             tc.psum_pool(name="apsum", bufs=1) as aps, \
             tc.psum_pool(name="apsA", bufs=3) as apsA:
            q3 = pp.tile([128, RLEN], BF16, name="q3")
            k3p = pp.tile([128, RLEN], BF16, name="k3p")
            k3m = pp.tile([128, RLEN], BF16, name="k3m")
            v3p = pp.tile([128, RLEN], BF16, name="v3p")
            v3m = pp.tile([128, RLEN], BF16, name="v3m")
            for t in (k3p, k3m, v3p, v3m):
                nc.gpsimd.dma_start(out=t[96:128, :], in_=ones_d.ap())
            for blk in range(NBLK):
                gbase = blk * BR * WP
                nc.gpsimd.dma_start(
                    out=q3[:, 0:PF + RMARG],
                    in_=fdram[0:32, 2 + gbase:2 + gbase + PF + RMARG]
                        .rearrange("c (u f) -> u c f", u=1)
                        .broadcast_to([4, 32, PF + RMARG]))
                xblk = ap1.tile([64, PF], F32, name="xblk", tag="xblk")
                for bi in range(2):
                    k3 = k3p if bi == 0 else k3m
                    v3 = v3p if bi == 0 else v3m
                    ksrc = fdram[32:64] if bi == 0 else sf_d.ap()[0:32]
                    vsrc = fdram[96:128] if bi == 0 else fdram[64:96]
                    for dx in range(3):
                        off = 2 + gbase + dx - 1
                        nc.gpsimd.dma_start(
                            out=k3[32 * dx:32 * dx + 32, 0:PF + RMARG],
                            in_=ksrc[:, off:off + PF + RMARG])
                        nc.gpsimd.dma_start(
                            out=v3[32 * dx:32 * dx + 32, 0:PF + RMARG],
                            in_=vsrc[:, off:off + PF + RMARG])
                    pt = []
                    for dy in range(3):
                        p = ap1.tile([128, PF], BF16, name=f"p{dy}", tag=f"p{dy}")
                        nc.vector.tensor_tensor(
                            out=p[:], in0=q3[:, WP:WP + PF],
                            in1=k3[:, dy * WP:dy * WP + PF], op=ALU.mult)
                        pt.append(p)
                    for c in range(NCH):
                        cb = c * CH
                        lps = aps.tile([72, CH], F32, name="lps", tag="lps")
                        for dy in range(3):
                            nc.tensor.matmul(
                                lps[:], ll[:, dy * 72:(dy + 1) * 72],
                                pt[dy][:, cb:cb + CH],
                                start=(dy == 0), stop=(dy == 2))
                        e = ap_.tile([72, CH], BF16, name="e", tag="e")
                        nc.scalar.activation(e[:], lps[:], AF.Exp)
                        s0p = aps.tile([8, CH], F32, name="s0p", tag="s0p")
                        nc.tensor.matmul(s0p[:], ls[:], e[:], start=True, stop=True)
                        rr = ap_.tile([8, CH], BF16, name="rr", tag="rr")
                        with nc.allow_low_precision(reason="softmax recip"):
                            nc.vector.reciprocal(rr[:], s0p[:])
                        r72 = aps.tile([72, CH], F32, name="r72", tag="r72")
                        nc.tensor.matmul(r72[:], lr[:], rr[:], start=True, stop=True)
                        at = ap_.tile([72, CH], BF16, name="at", tag="at")
                        nc.vector.tensor_tensor(out=at[:], in0=e[:], in1=r72[:],
                                                op=ALU.mult)
                        us = None
                        for dy in range(3):
                            ax = apsA.tile([128, CH], F32, name="ax", tag="ax")
                            nc.tensor.matmul(ax[:], la[:, dy * 128:(dy + 1) * 128],
                                             at[:], start=True, stop=True)
                            u = ap_.tile([128, CH], BF16, name=f"u{dy}", tag=f"u{dy}")
                            nc.vector.tensor_tensor(
                                out=u[:], in0=ax[:],
                                in1=v3[:, dy * WP + cb:dy * WP + cb + CH],
                                op=ALU.mult)
                            if us is None:
                                us = u
                            else:
                                dst = ap_.tile([128, CH], BF16, name="usum",
                                               tag="usum")
                                nc.vector.tensor_tensor(out=dst[:], in0=us[:],
                                                        in1=u[:], op=ALU.add)
                                us = dst
                        xps = aps.tile([32, CH], F32, name="xps", tag="xps")
                        nc.tensor.matmul(xps[:], lp[:, bi * 32:(bi + 1) * 32],
                                         us[:], start=True, stop=True)
                        nc.scalar.activation(
                            xblk[bi * 32:(bi + 1) * 32, cb:cb + CH], xps[:],
                            AF.Identity, bias=pb[bi * 32:(bi + 1) * 32, :])
                nc.sync.dma_start(
                    out=out_d.ap()[:, blk * BR * 256:(blk + 1) * BR * 256],
                    in_=xblk[:, 0:PGRID].rearrange("p (r w) -> p r w", r=BR)[:, :, 1:257])
    if not nc.is_finalized():
        nc.finalize()
    _CACHE["nc"] = nc
    return nc


# ---------------------------------------------------------------- entry
def kernel(x, ms, lpan, pan, s, w_q, w_kpan, w_vpan, w_kvms, w_dep, b_dep,
           w_proj_pan, b_proj_pan, w_proj_ms, b_proj_ms):
    x, ms, lpan, pan = [np.asarray(t, np.float32) for t in (x, ms, lpan, pan)]
    s = np.asarray(s, np.float32)
    LL, Ls, LR, LA, LP, pbias = _attn_weights(
        np.asarray(w_dep, np.float32), np.asarray(b_dep, np.float32),
        np.asarray(w_proj_pan, np.float32), np.asarray(b_proj_pan, np.float32),
        np.asarray(w_proj_ms, np.float32), np.asarray(b_proj_ms, np.float32))
    bf = ml_dtypes.bfloat16
    common = {
        "ones": _np(np.ones((32, RLEN), bf)),
        "lhsT_L": _np(LL.transpose(1, 0, 2).reshape(128, -1).astype(bf)),
        "lhsT_s": _np(Ls.astype(bf)),
        "lhsT_R": _np(LR.astype(bf)),
        "lhsT_A": _np(LA.transpose(1, 0, 2).reshape(72, -1).astype(bf)),
        "lhsT_P": _np(LP.transpose(1, 0, 2).reshape(128, -1).astype(bf)),
        "pbias": _np(pbias.reshape(64, 1)),
    }
    in_maps = []
    for core in range(8):
        b, r0 = core // 4, (core % 4) * 64
        lm = _fold_main_weights(np.asarray(w_q, np.float32),
                                np.asarray(w_kvms, np.float32),
                                np.asarray(w_vpan, np.float32), float(s[b]))
        xinp = np.zeros((51, 68, WP), np.float32)
        lo, hi = max(0, r0 - 2), min(256, r0 + 66)
        sl = np.s_[lo:hi]
        o = lo - (r0 - 2)
        n = hi - lo
        xinp[0:32, o:o + n, 1:257] = x[b][:, sl]
        xinp[32:40, o:o + n, 1:257] = ms[b][:, sl]
        xinp[40, o:o + n, 1:257] = lpan[b, 0, sl]
        xinp[41:49, o:o + n, 1:257] = ms[b][:, sl]
        xinp[49, o:o + n, 1:257] = lpan[b, 0, sl]
        xinp[50, o:o + n, 1:257] = pan[b, 0, sl]
        sf = _host_sfield(x, ms, np.asarray(w_kvms, np.float32), b, r0)
        m = dict(common)
        rm = np.ones((128, 2), np.float32)
        if r0 == 0:
            rm[:, 0] = 0.0
        if r0 == 192:
            rm[:, 1] = 0.0
        m["rowmask"] = _np(rm)
        m["xin"] = _np(xinp.reshape(51, -1).astype(bf))
        sfp = np.zeros((32, 2 + NF + 524), np.float32)
        sfp[:, 2:2 + NF] = sf.reshape(32, -1)
        m["sfield"] = _np(sfp.astype(bf))
        m["lhsT_main"] = _np(lm.transpose(1, 0, 2).reshape(51, -1).astype(bf))
        in_maps.append(m)

    nc = _build_nc()
    _CACHE["in_maps"] = in_maps
    res = run_bass_kernel_spmd(nc, in_maps, core_ids=list(range(8)))
    x_pan = np.zeros((B, 32, H, W), np.float32)
    x_ms = np.zeros((B, 32, H, W), np.float32)
    for core in range(8):
        b, r0 = core // 4, (core % 4) * 64
        o = res.results[core]["out"].reshape(64, 64, 256)
        x_pan[b, :, r0:r0 + 64] = o[0:32]
        x_ms[b, :, r0:r0 + 64] = o[32:64]
    return (x_pan, x_ms)



# revision 5
# speedup vs baseline: 1.4435x; 1.4435x over previous
"""Trainium2 Bass kernel for nn_CMAAA_29274497089816 (sparse local attention).

Sharding: data-parallel B(2) x H-slab(4) over 8 cores; each core computes
output rows [r0, r0+64) for both branches. The whole pipeline runs in SBUF:
  stage A: folded 3x3 conv -> fields F = [q, k_ms, v_ms, v_pan] (128 ch)
  stage B: k_ms conv on host-transposed column strips -> scatter into the
           scrambled S field (the reference's permute/reshape quirk)
  attention: 9-neighborhood softmax attention via matmuls with dx-stacked
             tiles, 16-row blocks, 512-px chunks; bf16 output.
Channel order inside each 32-group is (d, h) so the S scatter uses
contiguous partition ranges.
"""
import sys
sys.path.insert(0, "/opt/trn_rl_repo")
import numpy as np
import ml_dtypes

import concourse.bass as bass
import concourse.bacc as bacc
import concourse.mybir as mybir
from concourse import tile
from concourse.bass_utils import run_bass_kernel_spmd

BF16 = mybir.dt.bfloat16
F32 = mybir.dt.float32
AF = mybir.ActivationFunctionType
ALU = mybir.AluOpType

WP = 258
NF = 66 * WP                 # 17028 field px
XINW = 17552                 # xin dram width (1 zero + 68*WP + pad)
XIN3W = NF + 2               # 17030
SWIN = 20 * WP               # 5160 strip input px
XCOLW = 21164                # xcolT dram width (1 zero + 4*SWIN + pad)
XC3W = 4 * SWIN + 4          # 20644
SOW = 18 * WP                # 4644 strip output px
STW = 4 * SOW                # 18576
FW = 1 + NF + 3              # F tile width
SFW = 17808                  # S tile width (1 + NF + scatter margin)
BR = 16                      # output rows per attention block
NBLK = 4
BPX = BR * WP                # 4128
KW = (BR + 2) * WP           # 4644 stack width
CH = 512
# wpack column offsets
MAIN, KMS, LB, SB, RB, AB, PB, PH, WPW = 0, 384, 480, 696, 704, 776, 1160, 1224, 1296
SCALE = 0.5                  # hd ** -0.5
PERM = np.array([h * 4 + d for d in range(4) for h in range(8)])  # c_new -> c_old

_CACHE = {}


def _np(a):
    return np.ascontiguousarray(a)


# ---------------------------------------------------------------- host folds
def _fold_main(w_q, w_kvms, w_vpan, sb):
    """[126, 384]: rows (dy,ch[42]), cols (dx,out[128]); out blocks (d,h)."""
    L = np.zeros((3, 42, 3, 128), np.float32)
    for dy in range(3):
        for dx in range(3):
            Wq = w_q[:, :, dy, dx]
            Wk = w_kvms[:, :, dy, dx]
            Wv = w_vpan[:, :, dy, dx]
            L[dy, 0:32, dx, 0:32] = Wq[:, 0:32].T * SCALE
            L[dy, 32:40, dx, 0:32] = Wq[:, 32:40].T * SCALE * sb
            L[dy, 40, dx, 0:32] = Wq[:, 32:40].sum(1) * SCALE * (1.0 - sb)
            L[dy, 0:32, dx, 32:64] = Wk[0:32, 0:32].T
            L[dy, 32:40, dx, 32:64] = Wk[0:32, 32:40].T
            L[dy, 0:32, dx, 64:96] = Wk[32:64, 0:32].T
            L[dy, 32:40, dx, 64:96] = Wk[32:64, 32:40].T
            L[dy, 0:32, dx, 96:128] = Wv[:, 0:32].T
            L[dy, 40, dx, 96:128] = Wv[:, 32] - Wv[:, 34]
            L[dy, 41, dx, 96:128] = Wv[:, 33] + Wv[:, 34]
    L = L.reshape(3, 42, 3, 4, 32)[:, :, :, :, PERM].reshape(3, 42, 384)
    return L.reshape(126, 384)


def _fold_kms(w_kvms):
    """[120, 96]: rows (kx,ch[40]), cols (ky,out[32]); strip layout (c,w,y)."""
    L = np.zeros((3, 40, 3, 32), np.float32)
    for kx in range(3):
        for ky in range(3):
            L[kx, :, ky, :] = w_kvms[0:32, :, ky, kx].T
    return L[:, :, :, PERM].reshape(120, 96)


def _fold_attn(w_dep, b_dep, w_proj_pan, w_proj_ms):
    Wd = np.zeros((4, 9, 9), np.float32)
    for d in range(4):
        for j in range(9):
            Wd[d, :, j] = w_dep[d * 9 + j, 0].reshape(9)
    bd = b_dep.reshape(4, 9)
    L_L = np.zeros((3, 128, 72), np.float32)
    L_A = np.zeros((3, 72, 128), np.float32)
    for dy in range(3):
        for dx in range(3):
            t = dy * 3 + dx
            for h in range(8):
                for d in range(4):
                    L_L[dy, dx * 32 + d * 8 + h, h * 9:(h + 1) * 9] = Wd[d, t]
                    L_A[dy, h * 9:(h + 1) * 9, dx * 32 + d * 8 + h] = Wd[d, t]
    for h in range(8):
        for d in range(4):
            L_L[1, 96 + d * 8 + h, h * 9:(h + 1) * 9] = bd[d]
            L_A[1, h * 9:(h + 1) * 9, 96 + d * 8 + h] = bd[d]
    L_s = np.zeros((72, 8), np.float32)
    L_R = np.zeros((8, 72), np.float32)
    for h in range(8):
        L_s[h * 9:(h + 1) * 9, h] = 1.0
        L_R[h, h * 9:(h + 1) * 9] = 1.0
    P_lo = np.zeros((96, 64), np.float32)
    P_hi = np.zeros((32, 64), np.float32)
    for bi, wp in enumerate([w_proj_pan, w_proj_ms]):
        wt = wp[:, :, 0, 0].T[PERM]
        for dx in range(3):
            P_lo[dx * 32:(dx + 1) * 32, bi * 32:(bi + 1) * 32] = wt
        P_hi[:, bi * 32:(bi + 1) * 32] = wt
    return L_L, L_s, L_R, L_A, P_lo, P_hi


# ---------------------------------------------------------------- bass build
def _build_nc():
    if "nc" in _CACHE:
        return _CACHE["nc"]
    nc = bacc.Bacc(None, target_bir_lowering=False)
    xin_d = nc.declare_dram_parameter("xin", [42, XINW], BF16, isOutput=False)
    xc_d = nc.declare_dram_parameter("xcolT", [40, XCOLW], BF16, isOutput=False)
    wp_d = nc.declare_dram_parameter("wpack", [128, WPW], BF16, isOutput=False)
    ax_d = nc.declare_dram_parameter("aux", [128, 4], F32, isOutput=False)
    out_d = nc.declare_dram_parameter("out", [64, 64 * 256], BF16, isOutput=True)

    with tile.TileContext(nc) as tc:
      with tc.sbuf_pool(name="persist", bufs=1) as pp:
        wp = pp.tile([128, WPW], BF16, name="wp")
        nc.sync.dma_start(out=wp[:], in_=wp_d.ap())
        ax = pp.tile([128, 4], F32, name="ax")
        nc.sync.dma_start(out=ax[:], in_=ax_d.ap())
        F = pp.tile([128, FW], BF16, name="F")
        S = pp.tile([32, SFW], BF16, name="S")

        with tc.sbuf_pool(name="convin", bufs=1) as ci:
            xin3 = ci.tile([126, XIN3W], BF16, name="xin3")
            for dy in range(3):
                nc.sync.dma_start(out=xin3[dy * 42:(dy + 1) * 42, :],
                                  in_=xin_d.ap()[:, dy * WP: dy * WP + XIN3W])
            xc3 = ci.tile([120, XC3W], BF16, name="xc3")
            for kx in range(3):
                nc.gpsimd.dma_start(out=xc3[kx * 40:(kx + 1) * 40, :],
                                    in_=xc_d.ap()[:, kx * WP: kx * WP + XC3W])
            kT = ci.tile([32, STW], BF16, name="kT")
            nc.gpsimd.memset(S[:, :], 0.0)
            nc.vector.memset(F[:, 0:1], 0.0)
            nc.vector.memset(F[:, 1 + NF:FW], 0.0)

            with tc.psum_pool(name="cps", bufs=4) as cps:
                # ---- stage A: main conv -> F
                for c0 in range(0, NF, CH):
                    n = min(CH, NF - c0)
                    ps = cps.tile([128, CH], F32, name="psA", tag="psA")
                    for dx in range(3):
                        nc.tensor.matmul(
                            ps[:, 0:n],
                            wp[0:126, MAIN + dx * 128: MAIN + (dx + 1) * 128],
                            xin3[:, c0 + dx: c0 + dx + n],
                            start=(dx == 0), stop=(dx == 2))
                    nc.vector.tensor_copy(F[:, 1 + c0: 1 + c0 + n], ps[:, 0:n])
                    # zero the width-pad columns (y==0 / y==257 of each row)
                    w = ((c0 + WP - 1) // WP) * WP
                    while w < c0 + n:
                        nc.vector.memset(F[:, 1 + w: 2 + w], 0.0)
                        if w + WP - 1 < c0 + n:
                            nc.vector.memset(F[:, w + WP: w + WP + 1], 0.0)
                        w += WP
                    wl = ((c0 + WP - 1) // WP) * WP - 1
                    if c0 <= wl < c0 + n:
                        nc.vector.memset(F[:, 1 + wl: 2 + wl], 0.0)
                # out-of-image top/bottom field rows
                nc.vector.tensor_scalar_mul(F[:, 1:1 + WP], F[:, 1:1 + WP],
                                            ax[:, 0:1])
                nc.vector.tensor_scalar_mul(F[:, 1 + 65 * WP:1 + NF],
                                            F[:, 1 + 65 * WP:1 + NF], ax[:, 1:2])

                # ---- stage B: k_ms strips (transposed layout)
                for sp in range(4):
                    for c0 in range(0, SOW, CH):
                        n = min(CH, SOW - c0)
                        ps = cps.tile([32, CH], F32, name="psB", tag="psB")
                        for ky in range(3):
                            nc.tensor.matmul(
                                ps[:, 0:n],
                                wp[0:120, KMS + ky * 32: KMS + (ky + 1) * 32],
                                xc3[:, sp * SWIN + c0 + ky: sp * SWIN + c0 + ky + n],
                                start=(ky == 0), stop=(ky == 2))
                        nc.vector.tensor_copy(kT[:, sp * SOW + c0: sp * SOW + c0 + n],
                                              ps[:, 0:n])

            # ---- scatter strips into S (X = 4*w_rel + d - 3 rows)
            for dp in range(4):
                for d in range(4):
                    o0, no = (1, 17) if d == 0 else \
                             ((0, 17) if d == 3 else (1, 16))
                    row0 = 4 * o0 + d - 3
                    src = kT[d * 8:(d + 1) * 8,
                             dp * SOW + o0 * WP: dp * SOW + (o0 + no) * WP] \
                        .rearrange("p (r w) -> p r w", w=WP)[:, :, 1:257]
                    dst = S[dp * 8:(dp + 1) * 8,
                            1 + row0 * WP: 1 + row0 * WP + no * 4 * WP] \
                        .rearrange("p (r w) -> p r w", w=4 * WP)[:, :, 1:257]
                    nc.gpsimd.dma_start(out=dst, in_=src)
            nc.vector.tensor_scalar_mul(S[:, 1:1 + WP], S[:, 1:1 + WP],
                                        ax[0:32, 0:1])
            nc.vector.tensor_scalar_mul(S[:, 1 + 65 * WP:1 + NF],
                                        S[:, 1 + 65 * WP:1 + NF], ax[0:32, 1:2])

        # ---- attention
        with tc.sbuf_pool(name="stk", bufs=2) as sk, \
             tc.sbuf_pool(name="chk", bufs=2) as ck, \
             tc.sbuf_pool(name="xop", bufs=1) as xop, \
             tc.psum_pool(name="apsL", bufs=2) as apsL, \
             tc.psum_pool(name="apsS", bufs=1) as apsS, \
             tc.psum_pool(name="apsR", bufs=1) as apsR, \
             tc.psum_pool(name="apsA", bufs=3) as apsA, \
             tc.psum_pool(name="apsX", bufs=1) as apsX:
            for blk in range(NBLK):
                f0 = (1 + blk * BR) * WP
                q3 = sk.tile([128, BPX], BF16, name="q3", tag="q3")
                for g in range(4):
                    eng = nc.sync if g % 2 == 0 else nc.gpsimd
                    eng.dma_start(out=q3[g * 32:(g + 1) * 32, :],
                                  in_=F[0:32, 1 + f0: 1 + f0 + BPX])
                stacks = {}
                for nm, srct, p0 in (("k3p", F, 32), ("v3m", F, 64),
                                     ("v3p", F, 96), ("k3m", S, 0)):
                    t = sk.tile([96, KW], BF16, name=nm, tag=nm)
                    for dx in range(3):
                        eng = nc.sync if (dx + p0) % 2 == 0 else nc.gpsimd
                        eng.dma_start(
                            out=t[dx * 32:(dx + 1) * 32, :],
                            in_=srct[p0:p0 + 32, f0 - WP + dx: f0 - WP + dx + KW])
                    stacks[nm] = t
                xo = xop.tile([64, BPX], BF16, name="xo", tag="xo")
                for bi in range(2):
                    k3 = stacks["k3p"] if bi == 0 else stacks["k3m"]
                    v3 = stacks["v3p"] if bi == 0 else stacks["v3m"]
                    for c0 in range(0, BPX, CH):
                        n = min(CH, BPX - c0)
                        pt = []
                        for dy in range(3):
                            p = ck.tile([128, CH], BF16, name=f"p{dy}",
                                        tag=f"p{dy}")
                            nc.gpsimd.tensor_tensor(
                                out=p[0:96, 0:n], in0=q3[0:96, c0:c0 + n],
                                in1=k3[0:96, c0 + dy * WP: c0 + dy * WP + n],
                                op=ALU.mult)
                            pt.append(p)
                        nc.scalar.copy(pt[1][96:128, 0:n], q3[96:128, c0:c0 + n])
                        lps = apsL.tile([72, CH], F32, name="lps", tag="lps")
                        nc.tensor.matmul(lps[:, 0:n], wp[0:96, LB:LB + 72],
                                         pt[0][0:96, 0:n], start=True, stop=False)
                        nc.tensor.matmul(lps[:, 0:n], wp[0:128, LB + 72:LB + 144],
                                         pt[1][:, 0:n], start=False, stop=False)
                        nc.tensor.matmul(lps[:, 0:n], wp[0:96, LB + 144:LB + 216],
                                         pt[2][0:96, 0:n], start=False, stop=True)
                        e = ck.tile([72, CH], BF16, name="e", tag="e")
                        nc.scalar.activation(e[:, 0:n], lps[:, 0:n], AF.Exp)
                        s0p = apsS.tile([8, CH], F32, name="s0p", tag="s0p")
                        nc.tensor.matmul(s0p[:, 0:n], wp[0:72, SB:SB + 8],
                                         e[:, 0:n], start=True, stop=True)
                        rr = ck.tile([8, CH], BF16, name="rr", tag="rr")
                        with nc.allow_low_precision(reason="softmax recip"):
                            nc.vector.reciprocal(rr[:, 0:n], s0p[:, 0:n])
                        r72 = apsR.tile([72, CH], F32, name="r72", tag="r72")
                        nc.tensor.matmul(r72[:, 0:n], wp[0:8, RB:RB + 72],
                                         rr[:, 0:n], start=True, stop=True)
                        at = ck.tile([72, CH], BF16, name="at", tag="at")
                        nc.vector.tensor_tensor(out=at[:, 0:n], in0=e[:, 0:n],
                                                in1=r72[:, 0:n], op=ALU.mult)
                        # dy=1 first (bias rows live in ax1)
                        ax1 = apsA.tile([128, CH], F32, name="ax1", tag="ax")
                        nc.tensor.matmul(ax1[:, 0:n], wp[0:72, AB + 128:AB + 256],
                                         at[:, 0:n], start=True, stop=True)
                        us = ck.tile([96, CH], BF16, name="us", tag="us")
                        nc.vector.tensor_tensor(
                            out=us[:, 0:n], in0=ax1[0:96, 0:n],
                            in1=v3[0:96, c0 + WP: c0 + WP + n], op=ALU.mult)
                        ub = ck.tile([32, CH], BF16, name="ub", tag="ub")
                        nc.scalar.copy(ub[:, 0:n], ax1[96:128, 0:n])
                        for dy in (0, 2):
                            axp = apsA.tile([128, CH], F32, name=f"ax{dy}",
                                            tag="ax")
                            nc.tensor.matmul(
                                axp[0:96, 0:n],
                                wp[0:72, AB + dy * 128: AB + dy * 128 + 96],
                                at[:, 0:n], start=True, stop=True)
                            u = ck.tile([96, CH], BF16, name=f"u{dy}",
                                        tag=f"u{dy}")
                            nc.vector.tensor_tensor(
                                out=u[:, 0:n], in0=axp[0:96, 0:n],
                                in1=v3[0:96, c0 + dy * WP: c0 + dy * WP + n],
                                op=ALU.mult)
                            us2 = ck.tile([96, CH], BF16, name=f"us{dy}",
                                          tag=f"us{dy}")
                            nc.gpsimd.tensor_tensor(out=us2[:, 0:n],
                                                    in0=us[:, 0:n],
                                                    in1=u[:, 0:n], op=ALU.add)
                            us = us2
                        xps = apsX.tile([32, CH], F32, name="xps", tag="xps")
                        nc.tensor.matmul(xps[:, 0:n],
                                         wp[0:96, PB + bi * 32: PB + (bi + 1) * 32],
                                         us[:, 0:n], start=True, stop=False)
                        nc.tensor.matmul(xps[:, 0:n],
                                         wp[0:32, PH + bi * 32: PH + (bi + 1) * 32],
                                         ub[:, 0:n], start=False, stop=True)
                        nc.scalar.activation(
                            xo[bi * 32:(bi + 1) * 32, c0:c0 + n], xps[:, 0:n],
                            AF.Identity, bias=ax[bi * 32:(bi + 1) * 32, 2:3])
                nc.sync.dma_start(
                    out=out_d.ap()[:, blk * BR * 256:(blk + 1) * BR * 256],
                    in_=xo[:, 0:BPX].rearrange("p (r w) -> p r w", w=WP)[:, :, 1:257])
    if not nc.is_finalized():
        nc.finalize()
    _CACHE["nc"] = nc
    return nc


# ---------------------------------------------------------------- entry
def kernel(x, ms, lpan, pan, s, w_q, w_kpan, w_vpan, w_kvms, w_dep, b_dep,
           w_proj_pan, b_proj_pan, w_proj_ms, b_proj_ms):
    bf = ml_dtypes.bfloat16
    x, ms, lpan, pan = [np.asarray(t, np.float32) for t in (x, ms, lpan, pan)]
    s = np.asarray(s, np.float32)
    w_q, w_vpan, w_kvms = [np.asarray(t, np.float32) for t in (w_q, w_vpan, w_kvms)]
    L_L, L_s, L_R, L_A, P_lo, P_hi = _fold_attn(
        np.asarray(w_dep, np.float32), np.asarray(b_dep, np.float32),
        np.asarray(w_proj_pan, np.float32), np.asarray(w_proj_ms, np.float32))
    Lk = _fold_kms(w_kvms)

    # wpack shared across cores of one batch
    wpacks = []
    for b in range(2):
        W = np.zeros((128, WPW), np.float32)
        W[0:126, MAIN:MAIN + 384] = _fold_main(w_q, w_kvms, w_vpan, float(s[b]))
        W[0:120, KMS:KMS + 96] = Lk
        W[:, LB:LB + 216] = L_L.transpose(1, 0, 2).reshape(128, 216)
        W[0:72, SB:SB + 8] = L_s
        W[0:8, RB:RB + 72] = L_R
        W[0:72, AB:AB + 384] = L_A.transpose(1, 0, 2).reshape(72, 384)
        W[0:96, PB:PB + 64] = P_lo
        W[0:32, PH:PH + 64] = P_hi
        wpacks.append(_np(W.astype(bf)))
    pb = np.concatenate([np.asarray(b_proj_pan, np.float32),
                         np.asarray(b_proj_ms, np.float32)])

    in_maps = []
    for core in range(8):
        b, r0 = core // 4, (core % 4) * 64
        xin = np.zeros((42, XINW), np.float32)
        xs = xin[:, 1:1 + 68 * WP].reshape(42, 68, WP)
        lo, hi = max(0, r0 - 2), min(256, r0 + 66)
        o = lo - (r0 - 2)
        nn = hi - lo
        xs[0:32, o:o + nn, 1:257] = x[b][:, lo:hi]
        xs[32:40, o:o + nn, 1:257] = ms[b][:, lo:hi]
        xs[40, o:o + nn, 1:257] = lpan[b, 0, lo:hi]
        xs[41, o:o + nn, 1:257] = pan[b, 0, lo:hi]

        xc = np.zeros((40, XCOLW), np.float32)
        xcs = xc[:, 1:1 + 4 * SWIN].reshape(40, 4, 20, WP)
        for dp in range(4):
            w0 = 64 * dp + r0 // 4 - 2
            a, bb = max(0, w0), min(256, w0 + 20)
            if a < bb:
                xcs[0:32, dp, a - w0:bb - w0, 1:257] = \
                    x[b][:, :, a:bb].transpose(0, 2, 1)
                xcs[32:40, dp, a - w0:bb - w0, 1:257] = \
                    ms[b][:, :, a:bb].transpose(0, 2, 1)

        aux = np.zeros((128, 4), np.float32)
        aux[:, 0] = 0.0 if r0 == 0 else 1.0
        aux[:, 1] = 0.0 if r0 == 192 else 1.0
        aux[0:64, 2] = pb
        in_maps.append({
            "xin": _np(xin.astype(bf)),
            "xcolT": _np(xc.astype(bf)),
            "wpack": wpacks[b],
            "aux": _np(aux),
        })

    nc = _build_nc()
    _CACHE["in_maps"] = in_maps
    res = run_bass_kernel_spmd(nc, in_maps, core_ids=list(range(8)))
    x_pan = np.zeros((2, 32, 256, 256), np.float32)
    x_ms = np.zeros((2, 32, 256, 256), np.float32)
    for core in range(8):
        b, r0 = core // 4, (core % 4) * 64
        ob = np.asarray(res.results[core]["out"]).astype(np.float32) \
            .reshape(64, 64, 256)
        x_pan[b, :, r0:r0 + 64] = ob[0:32]
        x_ms[b, :, r0:r0 + 64] = ob[32:64]
    return (x_pan, x_ms)


# revision 8
# speedup vs baseline: 1.8820x; 1.3038x over previous
"""Trainium2 Bass kernel for nn_CMAAA_29274497089816 (sparse local attention).

Sharding: data-parallel B(2) x H-slab(4) over 8 cores; each core computes
output rows [r0, r0+64) for both branches. All-SBUF pipeline:
  stage A: folded 3x3 conv -> fields F = [q, k_ms, v_ms, v_pan] (128 ch)
  stage B: k_ms conv on host-transposed column strips -> scatter into the
           scrambled S field (the reference's permute/reshape quirk)
  attention: 9-neighborhood softmax attention via matmuls; dx-stacked tiles
             built with pad-skipping DMA patterns (no per-chunk memsets);
             block-wide p-products; branch-interleaved 512-px chunks.
Channel order inside each 32-group is (d, h) so the S scatter uses
contiguous partition ranges. Output is bf16.
"""
import sys
sys.path.insert(0, "/opt/trn_rl_repo")
import numpy as np
import ml_dtypes

import concourse.bass as bass
import concourse.bacc as bacc
import concourse.mybir as mybir
from concourse import tile
from concourse.bass_utils import run_bass_kernel_spmd

BF16 = mybir.dt.bfloat16
F32 = mybir.dt.float32
AF = mybir.ActivationFunctionType
ALU = mybir.AluOpType

WP = 258
NF = 66 * WP                 # 17028 field px
XINW = 17552                 # xin dram width (1 zero + 68*WP + pad)
XIN3W = NF + 2               # 17030
SWIN = 20 * WP               # 5160 strip input px
XCOLW = 21164                # xcolT dram width (1 zero + 4*SWIN + pad)
XC3W = 4 * SWIN + 4          # 20644
SOW = 18 * WP                # 4644 strip output px
STW = 4 * SOW                # 18576
FW = 1 + NF + 3              # F tile width
SFW = 17808                  # S tile width (1 + NF + scatter margin)
BR = 16                      # output rows per attention block
NBLK = 4
BPX = BR * WP                # 4128
KW = (BR + 2) * WP           # 4644 stack read width
KWA = KW + 2                 # stack tile alloc width (rearrange alignment)
CH = 512
# wpack column offsets
MAIN, KMS, LB, SB, RB, AB, PBM, WPW = 0, 384, 480, 696, 704, 776, 1160, 1232
SCALE = 0.5                  # hd ** -0.5
PERM = np.array([h * 4 + d for d in range(4) for h in range(8)])  # c_new -> c_old

_CACHE = {}


def _np(a):
    return np.ascontiguousarray(a)


# ---------------------------------------------------------------- host folds
def _fold_main(w_q, w_kvms, w_vpan, sb):
    """[126, 384]: rows (dy,ch[42]), cols (dx,out[128]); out blocks (d,h)."""
    L = np.zeros((3, 42, 3, 128), np.float32)
    for dy in range(3):
        for dx in range(3):
            Wq = w_q[:, :, dy, dx]
            Wk = w_kvms[:, :, dy, dx]
            Wv = w_vpan[:, :, dy, dx]
            L[dy, 0:32, dx, 0:32] = Wq[:, 0:32].T * SCALE
            L[dy, 32:40, dx, 0:32] = Wq[:, 32:40].T * SCALE * sb
            L[dy, 40, dx, 0:32] = Wq[:, 32:40].sum(1) * SCALE * (1.0 - sb)
            L[dy, 0:32, dx, 32:64] = Wk[0:32, 0:32].T
            L[dy, 32:40, dx, 32:64] = Wk[0:32, 32:40].T
            L[dy, 0:32, dx, 64:96] = Wk[32:64, 0:32].T
            L[dy, 32:40, dx, 64:96] = Wk[32:64, 32:40].T
            L[dy, 0:32, dx, 96:128] = Wv[:, 0:32].T
            L[dy, 40, dx, 96:128] = Wv[:, 32] - Wv[:, 34]
            L[dy, 41, dx, 96:128] = Wv[:, 33] + Wv[:, 34]
    L = L.reshape(3, 42, 3, 4, 32)[:, :, :, :, PERM].reshape(3, 42, 384)
    return L.reshape(126, 384)


def _fold_kms(w_kvms):
    """[120, 96]: rows (kx,ch[40]), cols (ky,out[32]); strip layout (c,w,y)."""
    L = np.zeros((3, 40, 3, 32), np.float32)
    for kx in range(3):
        for ky in range(3):
            L[kx, :, ky, :] = w_kvms[0:32, :, ky, kx].T
    return L[:, :, :, PERM].reshape(120, 96)


def _fold_attn(w_dep, b_dep, w_proj_pan, w_proj_ms):
    Wd = np.zeros((4, 9, 9), np.float32)
    for d in range(4):
        for j in range(9):
            Wd[d, :, j] = w_dep[d * 9 + j, 0].reshape(9)
    bd = b_dep.reshape(4, 9)
    L_L = np.zeros((3, 128, 72), np.float32)
    L_A = np.zeros((3, 72, 128), np.float32)
    for dy in range(3):
        for dx in range(3):
            t = dy * 3 + dx
            for h in range(8):
                for d in range(4):
                    L_L[dy, dx * 32 + d * 8 + h, h * 9:(h + 1) * 9] = Wd[d, t]
                    L_A[dy, h * 9:(h + 1) * 9, dx * 32 + d * 8 + h] = Wd[d, t]
    for h in range(8):
        for d in range(4):
            L_L[1, 96 + d * 8 + h, h * 9:(h + 1) * 9] = bd[d]
            L_A[1, h * 9:(h + 1) * 9, 96 + d * 8 + h] = bd[d]
    L_s = np.zeros((72, 8), np.float32)
    L_R = np.zeros((8, 72), np.float32)
    for h in range(8):
        L_s[h * 9:(h + 1) * 9, h] = 1.0
        L_R[h, h * 9:(h + 1) * 9] = 1.0
    P_m = np.zeros((128, 64), np.float32)
    for bi, wp in enumerate([w_proj_pan, w_proj_ms]):
        wt = wp[:, :, 0, 0].T[PERM]
        for dx in range(3):
            P_m[dx * 32:(dx + 1) * 32, bi * 32:(bi + 1) * 32] = wt
        P_m[96:128, bi * 32:(bi + 1) * 32] = wt
    return L_L, L_s, L_R, L_A, P_m


# ---------------------------------------------------------------- bass build
def _build_nc():
    if "nc" in _CACHE:
        return _CACHE["nc"]
    nc = bacc.Bacc(None, target_bir_lowering=False)
    xin_d = nc.declare_dram_parameter("xin", [42, XINW], BF16, isOutput=False)
    xc_d = nc.declare_dram_parameter("xcolT", [40, XCOLW], BF16, isOutput=False)
    wp_d = nc.declare_dram_parameter("wpack", [128, WPW], BF16, isOutput=False)
    ax_d = nc.declare_dram_parameter("aux", [128, 4], F32, isOutput=False)
    out_d = nc.declare_dram_parameter("out", [64, 64 * 256], BF16, isOutput=True)

    with tile.TileContext(nc) as tc:
      with tc.sbuf_pool(name="persist", bufs=1) as pp:
        wp = pp.tile([128, WPW], BF16, name="wp")
        nc.sync.dma_start(out=wp[:], in_=wp_d.ap())
        ax = pp.tile([128, 4], F32, name="ax")
        nc.sync.dma_start(out=ax[:], in_=ax_d.ap())
        F = pp.tile([128, FW], BF16, name="F")
        S = pp.tile([32, SFW], BF16, name="S")

        with tc.sbuf_pool(name="convin", bufs=1) as ci:
            xin3 = ci.tile([126, XIN3W], BF16, name="xin3")
            for dy in range(3):
                nc.sync.dma_start(out=xin3[dy * 42:(dy + 1) * 42, :],
                                  in_=xin_d.ap()[:, dy * WP: dy * WP + XIN3W])
            xc3 = ci.tile([120, XC3W], BF16, name="xc3")
            for kx in range(3):
                nc.gpsimd.dma_start(out=xc3[kx * 40:(kx + 1) * 40, :],
                                    in_=xc_d.ap()[:, kx * WP: kx * WP + XC3W])
            kT = ci.tile([32, STW], BF16, name="kT")
            nc.gpsimd.memset(S[:, :], 0.0)

            with tc.psum_pool(name="cps", bufs=4) as cps:
                # ---- stage A: main conv -> F (no pad zeroing; stacks skip pads)
                for c0 in list(range(0, NF - CH, CH)) + [NF - CH]:
                    ps = cps.tile([128, CH], F32, name="psA", tag="psA")
                    for dx in range(3):
                        nc.tensor.matmul(
                            ps[:],
                            wp[0:126, MAIN + dx * 128: MAIN + (dx + 1) * 128],
                            xin3[:, c0 + dx: c0 + dx + CH],
                            start=(dx == 0), stop=(dx == 2))
                    nc.vector.tensor_copy(F[:, 1 + c0: 1 + c0 + CH], ps[:])
                # out-of-image top/bottom field rows
                nc.vector.tensor_scalar_mul(F[:, 1:1 + WP], F[:, 1:1 + WP],
                                            ax[:, 0:1])
                nc.vector.tensor_scalar_mul(F[:, 1 + 65 * WP:1 + NF],
                                            F[:, 1 + 65 * WP:1 + NF], ax[:, 1:2])

                # ---- stage B: k_ms strips (transposed layout)
                for sp in range(4):
                    for c0 in list(range(0, SOW - CH, CH)) + [SOW - CH]:
                        ps = cps.tile([32, CH], F32, name="psB", tag="psB")
                        for ky in range(3):
                            nc.tensor.matmul(
                                ps[:],
                                wp[0:120, KMS + ky * 32: KMS + (ky + 1) * 32],
                                xc3[:, sp * SWIN + c0 + ky: sp * SWIN + c0 + ky + CH],
                                start=(ky == 0), stop=(ky == 2))
                        nc.vector.tensor_copy(
                            kT[:, sp * SOW + c0: sp * SOW + c0 + CH], ps[:])

            # ---- scatter strips into S (X = 4*w_rel + d - 3 rows)
            for dp in range(4):
                for d in range(4):
                    o0, no = (1, 17) if d == 0 else \
                             ((0, 17) if d == 3 else (1, 16))
                    row0 = 4 * o0 + d - 3
                    src = kT[d * 8:(d + 1) * 8,
                             dp * SOW + o0 * WP: dp * SOW + (o0 + no) * WP] \
                        .rearrange("p (r w) -> p r w", w=WP)[:, :, 1:257]
                    dst = S[dp * 8:(dp + 1) * 8,
                            1 + row0 * WP: 1 + row0 * WP + no * 4 * WP] \
                        .rearrange("p (r w) -> p r w", w=4 * WP)[:, :, 1:257]
                    nc.gpsimd.dma_start(out=dst, in_=src)
            nc.vector.tensor_scalar_mul(S[:, 1:1 + WP], S[:, 1:1 + WP],
                                        ax[0:32, 0:1])
            nc.vector.tensor_scalar_mul(S[:, 1 + 65 * WP:1 + NF],
                                        S[:, 1 + 65 * WP:1 + NF], ax[0:32, 1:2])

        # ---- attention
        with tc.sbuf_pool(name="stk", bufs=1) as sk, \
             tc.sbuf_pool(name="pwp", bufs=1) as pwpool, \
             tc.sbuf_pool(name="chk", bufs=2) as ck, \
             tc.sbuf_pool(name="xop", bufs=2) as xop, \
             tc.psum_pool(name="apsL", bufs=2) as apsL, \
             tc.psum_pool(name="apsS", bufs=1) as apsS, \
             tc.psum_pool(name="apsR", bufs=1) as apsR, \
             tc.psum_pool(name="apsA", bufs=3) as apsA, \
             tc.psum_pool(name="apsX", bufs=1) as apsX:
            for blk in range(NBLK):
                f0 = (1 + blk * BR) * WP
                q3 = sk.tile([128, BPX], BF16, name="q3", tag="q3")
                for g in range(4):
                    eng = nc.sync if g % 2 == 0 else nc.gpsimd
                    eng.dma_start(out=q3[g * 32:(g + 1) * 32, :],
                                  in_=F[0:32, 1 + f0: 1 + f0 + BPX])
                # k3m from S: S pads are already zero -> plain shifted copies
                k3m = sk.tile([96, KWA], BF16, name="k3m", tag="k3m")
                for dx in range(3):
                    eng = nc.sync if dx % 2 == 0 else nc.gpsimd
                    eng.dma_start(out=k3m[dx * 32:(dx + 1) * 32, 0:KW],
                                  in_=S[0:32, f0 - WP + dx: f0 - WP + dx + KW])
                # k3p/v3p/v3m from F: pad-skipping patterns into zeroed tiles
                stacks = {"k3m": k3m}
                for nm, p0 in (("k3p", 32), ("v3m", 64), ("v3p", 96)):
                    t = sk.tile([96, KWA], BF16, name=nm, tag=nm)
                    nc.gpsimd.memset(t[:, :], 0.0)
                    for dx in range(3):
                        dst = t[dx * 32:(dx + 1) * 32,
                                (2 - dx): (2 - dx) + 18 * WP] \
                            .rearrange("p (r w) -> p r w", w=WP)[:, :, 0:256]
                        src = F[p0:p0 + 32,
                                2 + blk * BR * WP: 2 + blk * BR * WP + 18 * WP] \
                            .rearrange("p (r w) -> p r w", w=WP)[:, :, 0:256]
                        eng = nc.sync if (dx + p0 // 32) % 2 == 0 else nc.gpsimd
                        eng.dma_start(out=dst, in_=src)
                    stacks[nm] = t
                # block-wide p products
                pws = {}
                for bi in range(2):
                    k3 = stacks["k3p"] if bi == 0 else stacks["k3m"]
                    for dy in range(3):
                        pw = pwpool.tile([128, BPX], BF16, name=f"pw{bi}{dy}",
                                         tag=f"pw{bi}{dy}")
                        nc.gpsimd.tensor_tensor(
                            out=pw[0:96, :], in0=q3[0:96, :],
                            in1=k3[0:96, dy * WP: dy * WP + BPX], op=ALU.mult)
                        if dy == 1:
                            nc.scalar.copy(pw[96:128, :], q3[96:128, :])
                        pws[(bi, dy)] = pw
                xo = xop.tile([64, BPX], BF16, name="xo", tag="xo")
                for c0 in list(range(0, BPX - CH, CH)) + [BPX - CH]:
                    for bi in range(2):
                        v3 = stacks["v3p"] if bi == 0 else stacks["v3m"]
                        lps = apsL.tile([72, CH], F32, name="lps", tag="lps")
                        nc.tensor.matmul(lps[:], wp[0:96, LB:LB + 72],
                                         pws[(bi, 0)][0:96, c0:c0 + CH],
                                         start=True, stop=False)
                        nc.tensor.matmul(lps[:], wp[0:128, LB + 72:LB + 144],
                                         pws[(bi, 1)][:, c0:c0 + CH],
                                         start=False, stop=False)
                        nc.tensor.matmul(lps[:], wp[0:96, LB + 144:LB + 216],
                                         pws[(bi, 2)][0:96, c0:c0 + CH],
                                         start=False, stop=True)
                        e = ck.tile([72, CH], BF16, name="e", tag="e")
                        nc.scalar.activation(e[:], lps[:], AF.Exp)
                        s0p = apsS.tile([8, CH], F32, name="s0p", tag="s0p")
                        nc.tensor.matmul(s0p[:], wp[0:72, SB:SB + 8], e[:],
                                         start=True, stop=True)
                        rr = ck.tile([8, CH], BF16, name="rr", tag="rr")
                        with nc.allow_low_precision(reason="softmax recip"):
                            nc.vector.reciprocal(rr[:], s0p[:])
                        r72 = apsR.tile([72, CH], F32, name="r72", tag="r72")
                        nc.tensor.matmul(r72[:], wp[0:8, RB:RB + 72], rr[:],
                                         start=True, stop=True)
                        at = ck.tile([72, CH], BF16, name="at", tag="at")
                        nc.vector.tensor_tensor(out=at[:], in0=e[:], in1=r72[:],
                                                op=ALU.mult)
                        ax1 = apsA.tile([128, CH], F32, name="ax1", tag="ax")
                        nc.tensor.matmul(ax1[:], wp[0:72, AB + 128:AB + 256],
                                         at[:], start=True, stop=True)
                        us1 = ck.tile([128, CH], BF16, name="us1", tag="us1")
                        nc.vector.tensor_tensor(
                            out=us1[0:96, :], in0=ax1[0:96, :],
                            in1=v3[0:96, c0 + WP: c0 + WP + CH], op=ALU.mult)
                        ax0 = apsA.tile([128, CH], F32, name="ax0", tag="ax")
                        nc.tensor.matmul(ax0[0:96, :], wp[0:72, AB:AB + 96],
                                         at[:], start=True, stop=True)
                        u0 = ck.tile([96, CH], BF16, name="u0", tag="u0")
                        nc.vector.tensor_tensor(
                            out=u0[:], in0=ax0[0:96, :],
                            in1=v3[0:96, c0: c0 + CH], op=ALU.mult)
                        us2 = ck.tile([128, CH], BF16, name="us2", tag="us2")
                        nc.gpsimd.tensor_tensor(out=us2[0:96, :], in0=us1[0:96, :],
                                                in1=u0[:], op=ALU.add)
                        ax2 = apsA.tile([128, CH], F32, name="ax2", tag="ax")
                        nc.tensor.matmul(ax2[0:96, :],
                                         wp[0:72, AB + 256:AB + 352],
                                         at[:], start=True, stop=True)
                        u2 = ck.tile([96, CH], BF16, name="u2", tag="u2")
                        nc.vector.tensor_tensor(
                            out=u2[:], in0=ax2[0:96, :],
                            in1=v3[0:96, c0 + 2 * WP: c0 + 2 * WP + CH],
                            op=ALU.mult)
                        us3 = ck.tile([128, CH], BF16, name="us3", tag="us3")
                        nc.gpsimd.tensor_tensor(out=us3[0:96, :], in0=us2[0:96, :],
                                                in1=u2[:], op=ALU.add)
                        nc.scalar.copy(us3[96:128, :], ax1[96:128, :])
                        xps = apsX.tile([32, CH], F32, name="xps", tag="xps")
                        nc.tensor.matmul(xps[:],
                                         wp[0:128, PBM + bi * 32: PBM + (bi + 1) * 32],
                                         us3[:], start=True, stop=True)
                        nc.scalar.activation(
                            xo[bi * 32:(bi + 1) * 32, c0:c0 + CH], xps[:],
                            AF.Identity, bias=ax[bi * 32:(bi + 1) * 32, 2:3])
                nc.sync.dma_start(
                    out=out_d.ap()[:, blk * BR * 256:(blk + 1) * BR * 256],
                    in_=xo[:, 0:BPX].rearrange("p (r w) -> p r w", w=WP)[:, :, 1:257])
    if not nc.is_finalized():
        nc.finalize()
    _CACHE["nc"] = nc
    return nc


# ---------------------------------------------------------------- entry
def kernel(x, ms, lpan, pan, s, w_q, w_kpan, w_vpan, w_kvms, w_dep, b_dep,
           w_proj_pan, b_proj_pan, w_proj_ms, b_proj_ms):
    bf = ml_dtypes.bfloat16
    x, ms, lpan, pan = [np.asarray(t, np.float32) for t in (x, ms, lpan, pan)]
    s = np.asarray(s, np.float32)
    w_q, w_vpan, w_kvms = [np.asarray(t, np.float32) for t in (w_q, w_vpan, w_kvms)]
    L_L, L_s, L_R, L_A, P_m = _fold_attn(
        np.asarray(w_dep, np.float32), np.asarray(b_dep, np.float32),
        np.asarray(w_proj_pan, np.float32), np.asarray(w_proj_ms, np.float32))
    Lk = _fold_kms(w_kvms)

    wpacks = []
    for b in range(2):
        W = np.zeros((128, WPW), np.float32)
        W[0:126, MAIN:MAIN + 384] = _fold_main(w_q, w_kvms, w_vpan, float(s[b]))
        W[0:120, KMS:KMS + 96] = Lk
        W[:, LB:LB + 216] = L_L.transpose(1, 0, 2).reshape(128, 216)
        W[0:72, SB:SB + 8] = L_s
        W[0:8, RB:RB + 72] = L_R
        W[0:72, AB:AB + 384] = L_A.transpose(1, 0, 2).reshape(72, 384)
        W[:, PBM:PBM + 64] = P_m
        wpacks.append(_np(W.astype(bf)))
    pb = np.concatenate([np.asarray(b_proj_pan, np.float32),
                         np.asarray(b_proj_ms, np.float32)])

    in_maps = []
    for core in range(8):
        b, r0 = core // 4, (core % 4) * 64
        xin = np.zeros((42, XINW), np.float32)
        xs = xin[:, 1:1 + 68 * WP].reshape(42, 68, WP)
        lo, hi = max(0, r0 - 2), min(256, r0 + 66)
        o = lo - (r0 - 2)
        nn = hi - lo
        xs[0:32, o:o + nn, 1:257] = x[b][:, lo:hi]
        xs[32:40, o:o + nn, 1:257] = ms[b][:, lo:hi]
        xs[40, o:o + nn, 1:257] = lpan[b, 0, lo:hi]
        xs[41, o:o + nn, 1:257] = pan[b, 0, lo:hi]

        xc = np.zeros((40, XCOLW), np.float32)
        xcs = xc[:, 1:1 + 4 * SWIN].reshape(40, 4, 20, WP)
        for dp in range(4):
            w0 = 64 * dp + r0 // 4 - 2
            a, bb = max(0, w0), min(256, w0 + 20)
            if a < bb:
                xcs[0:32, dp, a - w0:bb - w0, 1:257] = \
                    x[b][:, :, a:bb].transpose(0, 2, 1)
                xcs[32:40, dp, a - w0:bb - w0, 1:257] = \
                    ms[b][:, :, a:bb].transpose(0, 2, 1)

        aux = np.zeros((128, 4), np.float32)
        aux[:, 0] = 0.0 if r0 == 0 else 1.0
        aux[:, 1] = 0.0 if r0 == 192 else 1.0
        aux[0:64, 2] = pb
        in_maps.append({
            "xin": _np(xin.astype(bf)),
            "xcolT": _np(xc.astype(bf)),
            "wpack": wpacks[b],
            "aux": _np(aux),
        })

    nc = _build_nc()
    _CACHE["in_maps"] = in_maps
    res = run_bass_kernel_spmd(nc, in_maps, core_ids=list(range(8)))
    x_pan = np.zeros((2, 32, 256, 256), np.float32)
    x_ms = np.zeros((2, 32, 256, 256), np.float32)
    for core in range(8):
        b, r0 = core // 4, (core % 4) * 64
        ob = np.asarray(res.results[core]["out"]).astype(np.float32) \
            .reshape(64, 64, 256)
        x_pan[b, :, r0:r0 + 64] = ob[0:32]
        x_ms[b, :, r0:r0 + 64] = ob[32:64]
    return (x_pan, x_ms)
